# revision 1
# baseline (speedup 1.0000x reference)
"""DbrxAttention (GQA + RoPE + causal) on 8 Trainium2 NeuronCores.

Tensor-parallel over heads: core c owns q heads [6c, 6c+6) and kv head c.
Per core: QKV projection (transposed layout, bf16, weight-shared matmul
pairs), RoPE, causal attention (scores transposed: kv on partitions, q on
free dim; softmax denominator via ones-column matmul), AllToAll (split in
two halves, overlapped with attention) to redistribute attention outputs,
then each core computes a 256-row sequence block of the output projection
against the full w_out.

kernel(**inputs) takes the full unsharded inputs and returns the full output.
"""

import math

import numpy as np
import ml_dtypes

import concourse.bass as bass
import concourse.mybir as mybir
from concourse import bacc
import concourse.tile as tile
from concourse.bass_utils import run_bass_kernel_spmd
from concourse.masks import make_identity

BF16 = mybir.dt.bfloat16
F32 = mybir.dt.float32
NP_BF16 = ml_dtypes.bfloat16

# full-size problem config
B, S, D = 1, 2048, 6144
H, KV, HD = 48, 8, 128
R = 8  # cores


class Cfg:
    def __init__(self, S=2048, KO=48, NQ=6, R=8, DO=6144, IC=512, CH=512,
                 OT=512):
        self.S = S          # sequence length
        self.KO = KO        # contraction k-tiles for QKV (D = KO*128)
        self.NQ = NQ        # q heads per core
        self.R = R          # cores
        self.DO = DO        # out_proj output dim
        self.IC = IC        # attention i-chunk (free dim per scores matmul)
        self.CH = CH        # QKV s-chunk (pair of CH/2 matmuls)
        self.OT = OT        # out_proj n-chunk
        self.D = KO * 128
        self.SB = S // R    # seq block per core after AllToAll
        self.KO2 = R * NQ   # contraction k-tiles for out_proj (H*HD = KO2*128)
        self.NT = DO // OT
        assert S % R == 0 and S % IC == 0 and S % CH == 0 and DO % OT == 0
        assert IC % 128 == 0 and (IC // 128) % 2 == 0
        assert (NQ + 2) % 2 == 0 and CH % 2 == 0
        assert NQ % 2 == 0  # split-A2A halves


# e-tile order within the per-core QKV projection: k, v first so RoPE(k) and
# the v transpose can overlap the second projection half, and attention can
# start the moment the projection finishes.
def _e_order(NQ):
    return ["k", "v"] + [f"q{h}" for h in range(NQ)]


def build(cfg: Cfg, debug_taps: bool = False, split_a2a: bool = True,
          qkv_pair: bool = True, early_rope: bool = True) -> bacc.Bacc:
    S, KO, NQ, IC, CH = cfg.S, cfg.KO, cfg.NQ, cfg.IC, cfg.CH
    NE = NQ + 2            # qkv e-tiles per core
    EHALF = NE // 2
    NCH = S // CH
    NIC = S // IC
    ND = IC // 128         # diagonal j-tiles per i-chunk
    NJ = S // 128
    SB, KO2, NT, DO, OT = cfg.SB, cfg.KO2, cfg.NT, cfg.DO, cfg.OT
    CHH = CH // 2
    softmax_scale = 1.0 / math.sqrt(HD)
    ET_K, ET_V = 0, 1      # e-tile indices of k and v
    NQH = NQ // 2

    nc = bacc.Bacc("TRN2", target_bir_lowering=False, debug=False,
                   num_devices=cfg.R)

    hid_d = nc.dram_tensor("hid", [NCH, 128, KO, CH], BF16,
                           kind="ExternalInput")
    wq_d = nc.dram_tensor("wq", [2, 128, KO, EHALF * 128], BF16,
                          kind="ExternalInput")
    wo_d = nc.dram_tensor("wo", [NT, 128, KO2, OT], BF16,
                          kind="ExternalInput")
    cos_d = nc.dram_tensor("cosT", [128, S], BF16, kind="ExternalInput")
    sin_d = nc.dram_tensor("sinT", [128, S], BF16, kind="ExternalInput")
    msk_d = nc.dram_tensor("masks", [128, ND, IC], BF16, kind="ExternalInput")
    out_d = nc.dram_tensor("out", [SB, DO], F32, kind="ExternalOutput")
    if debug_taps:
        dbg_qkv = nc.dram_tensor("dbg_qkv", [128, NE, S], BF16,
                                 kind="ExternalOutput")
        dbg_oT = nc.dram_tensor("dbg_oT", [128, NQ, S], BF16,
                                kind="ExternalOutput")
        dbg_d = nc.dram_tensor("dbg_d", [NQ, S], F32, kind="ExternalOutput")
        dbg_otf = nc.dram_tensor("dbg_otf", [128, KO2, SB], BF16,
                                 kind="ExternalOutput")

    with (
        tile.TileContext(nc) as tc,
        tc.tile_pool(name="psum", bufs=2, space="PSUM") as psp,
        tc.tile_pool(name="dram", bufs=1, space="DRAM") as dram,
    ):
        with (
            tc.tile_pool(name="big", bufs=1) as big,
            tc.tile_pool(name="attc", bufs=1) as attc,
        ):
            qkv_sb = big.tile([128, NE, S], BF16)
            ones_sb = attc.tile([128, 1], BF16)
            nc.gpsimd.memset(ones_sb[:], 1.0)
            v_nat = attc.tile([128, NJ, 128], BF16)

            # ---- phase 1: QKV projection (transposed: [e, s]) + RoPE ----
            with (
                tc.tile_pool(name="ropec", bufs=1) as ropec,
                tc.tile_pool(name="wqp", bufs=1) as wqp,
                tc.tile_pool(name="hidp", bufs=2) as hidp,
                tc.tile_pool(name="ropep", bufs=1) as ropep,
            ):
                cos_sb = ropec.tile([128, S], BF16)
                nc.sync.dma_start(cos_sb[:], cos_d.ap())
                sin_sb = ropec.tile([128, S], BF16)
                nc.sync.dma_start(sin_sb[:], sin_d.ap())
                ident = ropec.tile([128, 128], BF16)
                make_identity(nc, ident[:])

                def rope_head(et):
                    # two free-dim halves to halve the rot scratch tile
                    for sh in range(2):
                        sl = slice(sh * (S // 2), (sh + 1) * (S // 2))
                        x = qkv_sb[:, et, sl]
                        rot = ropep.tile([128, S // 2], BF16, tag="rot",
                                         name=f"rot{et}_{sh}")
                        nc.scalar.copy(rot[0:64, :], x[64:128, :])
                        nc.scalar.copy(rot[64:128, :], x[0:64, :])
                        nc.vector.tensor_mul(rot[:], rot[:], sin_sb[:, sl])
                        nc.vector.tensor_mul(x, x, cos_sb[:, sl])
                        nc.vector.tensor_add(x, x, rot[:])

                def vT_head():
                    for st in range(NJ):
                        pt = psp.tile([128, 128], BF16, tag="pv",
                                      name=f"tp{st}")
                        nc.tensor.transpose(
                            pt[:], qkv_sb[:, ET_V, st * 128:(st + 1) * 128],
                            ident[:])
                        nc.vector.tensor_copy(v_nat[:, st, :], pt[:])

                for half in range(2):
                    wq_sb = wqp.tile([128, KO, EHALF * 128], BF16, tag="wq",
                                     name=f"wq{half}")
                    nc.sync.dma_start(wq_sb[:], wq_d.ap()[half])
                    for ci in range(NCH):
                        hid_t = hidp.tile([128, KO, CH], BF16, tag="hid",
                                          name=f"hid{half}_{ci}")
                        nc.sync.dma_start(hid_t[:], hid_d.ap()[ci])
                        for el in range(EHALF):
                            et = half * EHALF + el
                            ps = psp.tile([128, 1024], F32, tag="ps")
                            if qkv_pair:
                                for ko in range(KO):
                                    w = wq_sb[:, ko, el * 128:(el + 1) * 128]
                                    nc.tensor.matmul(
                                        ps[:, :CHH], lhsT=w,
                                        rhs=hid_t[:, ko, :CHH],
                                        start=(ko == 0), stop=(ko == KO - 1))
                                    nc.tensor.matmul(
                                        ps[:, 512:512 + CHH], lhsT=w,
                                        rhs=hid_t[:, ko, CHH:],
                                        start=(ko == 0), stop=(ko == KO - 1))
                                nc.vector.tensor_copy(
                                    qkv_sb[:, et, ci * CH:ci * CH + CHH],
                                    ps[:, :CHH])
                                nc.vector.tensor_copy(
                                    qkv_sb[:, et, ci * CH + CHH:(ci + 1) * CH],
                                    ps[:, 512:512 + CHH])
                            else:
                                for ko in range(KO):
                                    w = wq_sb[:, ko, el * 128:(el + 1) * 128]
                                    nc.tensor.matmul(
                                        ps[:, :CH], lhsT=w,
                                        rhs=hid_t[:, ko, :],
                                        start=(ko == 0), stop=(ko == KO - 1))
                                nc.vector.tensor_copy(
                                    qkv_sb[:, et, ci * CH:(ci + 1) * CH],
                                    ps[:, :CH])
                    # post-half epilogues (overlap the other half / attention)
                    if not early_rope:
                        continue
                    if half == 0:
                        rope_head(ET_K)
                        vT_head()
                        for el in range(2, EHALF):
                            rope_head(el)  # q0, q1 (et==2+h)
                    else:
                        for el in range(EHALF):
                            rope_head(EHALF + el)  # q2..q5
                if not early_rope:
                    rope_head(ET_K)
                    vT_head()
                    for h_ in range(NQ):
                        rope_head(2 + h_)

            # ---- phase 2+3: attention, normalize, split AllToAll ----
            a2a_in = [dram.tile([cfg.R, NQH * 128, SB], BF16,
                                name=f"a2a_in{i}") for i in range(2)]
            a2a_out = [dram.tile([cfg.R, NQH * 128, SB], BF16,
                                 name=f"a2a_out{i}") for i in range(2)]
            with (
                tc.tile_pool(name="attw", bufs=1) as attw,
                tc.tile_pool(name="pp", bufs=4) as pp,
                tc.tile_pool(name="dp", bufs=NQ) as dpool,
                tc.tile_pool(name="rp", bufs=2) as rp,
                tc.tile_pool(name="rbp", bufs=2) as rbp,
            ):
                msk_sb = attw.tile([128, ND, IC], BF16)
                nc.sync.dma_start(msk_sb[:], msk_d.ap())
                oT_sb = attw.tile([128, NQ, S], BF16)
                d_sb = [dpool.tile([1, S], F32, tag="d", name=f"d{h}")
                        for h in range(NQ)]
                kT = qkv_sb[:, ET_K, :]
                for h in range(NQ):
                    qT = qkv_sb[:, 2 + h, :]
                    for ci in range(NIC):
                        jt_max = (ci + 1) * ND
                        pv = psp.tile([128, 512], F32, tag="pv")
                        dq = psp.tile([1, 512], F32, tag="dq")
                        for jp in range(jt_max // 2):
                            sc = psp.tile([128, 1024], F32, tag="ps")
                            p2 = pp.tile([128, 1024], BF16, tag="p")
                            for u in range(2):
                                jt = 2 * jp + u
                                nc.tensor.matmul(
                                    sc[:, u * 512:u * 512 + IC],
                                    lhsT=kT[:, jt * 128:(jt + 1) * 128],
                                    rhs=qT[:, ci * IC:(ci + 1) * IC],
                                    start=True, stop=True)
                            nc.scalar.activation(
                                p2[:], sc[:],
                                mybir.ActivationFunctionType.Exp,
                                scale=softmax_scale)
                            for u in range(2):
                                jt = 2 * jp + u
                                pu = p2[:, u * 512:u * 512 + IC]
                                if jt >= ci * ND:
                                    nc.vector.tensor_mul(
                                        pu, pu, msk_sb[:, jt - ci * ND, :])
                                nc.tensor.matmul(
                                    pv[:, :IC], lhsT=v_nat[:, jt, :], rhs=pu,
                                    start=(jt == 0), stop=(jt == jt_max - 1))
                                nc.tensor.matmul(
                                    dq[:, :IC], lhsT=ones_sb[:, 0:1], rhs=pu,
                                    start=(jt == 0), stop=(jt == jt_max - 1))
                        nc.vector.tensor_copy(
                            oT_sb[:, h, ci * IC:(ci + 1) * IC], pv[:, :IC])
                        nc.vector.tensor_copy(
                            d_sb[h][:, ci * IC:(ci + 1) * IC], dq[:, :IC])
                    # normalize head h and ship it to its A2A buffer
                    r_t = rp.tile([1, S], F32, tag="r", name=f"r{h}")
                    nc.vector.reciprocal_approx_fast(r_t[:], d_sb[h][:])
                    for ci in range(NIC):
                        rb = rbp.tile([128, IC], F32, tag="rb")
                        nc.gpsimd.partition_broadcast(
                            rb[:], r_t[:, ci * IC:(ci + 1) * IC])
                        o = oT_sb[:, h, ci * IC:(ci + 1) * IC]
                        nc.vector.tensor_mul(o, o, rb[:])
                    grp, hl = divmod(h, NQH)
                    nc.sync.dma_start(
                        a2a_in[grp][:, hl * 128:(hl + 1) * 128, :]
                        .rearrange("r p s -> p r s"),
                        oT_sb[:, h, :].rearrange("p (r s) -> p r s", r=cfg.R))
                    if split_a2a and (h == NQH - 1 or h == NQ - 1):
                        grp = h // NQH
                        nc.gpsimd.collective_compute(
                            "AllToAll", mybir.AluOpType.bypass,
                            replica_groups=[list(range(cfg.R))],
                            ins=[a2a_in[grp][:]], outs=[a2a_out[grp][:]])
                if not split_a2a:
                    for grp in range(2):
                        nc.gpsimd.collective_compute(
                            "AllToAll", mybir.AluOpType.bypass,
                            replica_groups=[list(range(cfg.R))],
                            ins=[a2a_in[grp][:]], outs=[a2a_out[grp][:]])
                if debug_taps:
                    nc.sync.dma_start(dbg_qkv.ap(), qkv_sb[:])
                    nc.sync.dma_start(dbg_oT.ap(), oT_sb[:])
                    for h in range(NQ):
                        nc.sync.dma_start(dbg_d.ap()[h:h + 1, :], d_sb[h][:])

        # ---- phase 4: out_proj on this core's seq block ----
        with (
            tc.tile_pool(name="otf", bufs=1) as otf,
            tc.tile_pool(name="wop", bufs=2) as wop,
            tc.tile_pool(name="obp", bufs=2) as obp,
        ):
            oT_full = otf.tile([128, KO2, SB], BF16)
            for grp in range(2):
                for k3 in range(NQH):
                    nc.sync.dma_start(
                        oT_full[:].rearrange("p (r k6) s -> p k6 r s",
                                             r=cfg.R)[:, grp * NQH + k3],
                        a2a_out[grp][:, k3 * 128:(k3 + 1) * 128, :]
                        .rearrange("r p s -> p r s"))
            if debug_taps:
                nc.sync.dma_start(dbg_otf.ap(), oT_full[:])
            for nt in range(NT):
                wo_t = wop.tile([128, KO2, OT], BF16, tag="wo")
                nc.sync.dma_start(wo_t[:], wo_d.ap()[nt])
                for mi in range((SB + 127) // 128):
                    msz = min(128, SB - mi * 128)
                    ps = psp.tile([128, 1024], F32, tag="ps")
                    for ko in range(KO2):
                        nc.tensor.matmul(
                            ps[:msz, :OT],
                            lhsT=oT_full[:, ko, mi * 128:mi * 128 + msz],
                            rhs=wo_t[:, ko, :],
                            start=(ko == 0), stop=(ko == KO2 - 1))
                    ob = obp.tile([128, OT], F32, tag="ob")
                    nc.vector.tensor_copy(ob[:msz, :], ps[:msz, :OT])
                    nc.sync.dma_start(
                        out_d.ap()[mi * 128:mi * 128 + msz,
                                   nt * OT:(nt + 1) * OT],
                        ob[:msz, :])

    nc.compile()
    return nc


def make_masks(cfg: Cfg) -> np.ndarray:
    ND = cfg.IC // 128
    jj = np.arange(128)[:, None, None]
    rr = np.arange(ND)[None, :, None]
    ii = np.arange(cfg.IC)[None, None, :]
    return (jj + 128 * rr <= ii).astype(NP_BF16)


def shard_inputs(cfg: Cfg, hidden_states, cos, sin, w_qkv, w_out,
                 n_heads, n_kv):
    """Build per-core input maps (host-side shard + bf16 cast + layout)."""
    S, KO, NQ, R = cfg.S, cfg.KO, cfg.NQ, cfg.R
    D = cfg.D
    NCH, CH = S // cfg.CH, cfg.CH
    hid_T = np.ascontiguousarray(hidden_states.reshape(S, D).T)  # [D, S]
    # [NCH, 128, KO, CH]
    hid_l = (hid_T.reshape(KO, 128, NCH, CH).transpose(2, 1, 0, 3)
             .astype(NP_BF16))
    hid_l = np.ascontiguousarray(hid_l)
    NT, OT = cfg.NT, cfg.OT
    wo_l = (w_out.reshape(cfg.KO2, 128, NT, OT).transpose(2, 1, 0, 3)
            .astype(NP_BF16))
    wo_l = np.ascontiguousarray(wo_l)
    cos_T = cos.T.astype(NP_BF16)  # [HD, S]
    sin_T = sin.T
    sinS = np.concatenate([-sin_T[:64], sin_T[64:]], axis=0).astype(NP_BF16)
    masks = make_masks(cfg)

    in_maps = []
    NE = NQ + 2
    EHALF = NE // 2
    for c in range(R):
        qs = c * NQ * 128
        # e-tile order: k, v, q0..q5
        wsh = np.concatenate([
            w_qkv[:, n_heads * HD + c * 128: n_heads * HD + (c + 1) * 128],
            w_qkv[:, (n_heads + n_kv) * HD + c * 128:
                  (n_heads + n_kv) * HD + (c + 1) * 128],
            w_qkv[:, qs:qs + NQ * 128],
        ], axis=1)  # [D, NE*128]
        wq_l = (wsh.reshape(KO, 128, 2, EHALF * 128).transpose(2, 1, 0, 3)
                .astype(NP_BF16))
        in_maps.append({
            "hid": hid_l, "wq": np.ascontiguousarray(wq_l), "wo": wo_l,
            "cosT": cos_T, "sinT": sinS, "masks": masks,
        })
    return in_maps


_cached = {}


def _get_nc(cfg: Cfg, debug_taps: bool = False, **bkw):
    key = (tuple(sorted(cfg.__dict__.items())), debug_taps,
           tuple(sorted(bkw.items())))
    if key not in _cached:
        _cached[key] = build(cfg, debug_taps=debug_taps, **bkw)
    return _cached[key]


def run(cfg: Cfg, in_maps, debug_taps: bool = False, build_kwargs=None,
        **kwargs):
    nc = _get_nc(cfg, debug_taps, **(build_kwargs or {}))
    res = run_bass_kernel_spmd(nc, in_maps, core_ids=list(range(cfg.R)),
                               **kwargs)
    out = np.concatenate([res.results[c]["out"] for c in range(cfg.R)],
                         axis=0)
    return out, res


def kernel(hidden_states, cos, sin, w_qkv, w_out):
    cfg = Cfg()
    hidden_states = np.asarray(hidden_states, dtype=np.float32)
    cos = np.asarray(cos, dtype=np.float32)
    sin = np.asarray(sin, dtype=np.float32)
    w_qkv = np.asarray(w_qkv, dtype=np.float32)
    w_out = np.asarray(w_out, dtype=np.float32)
    in_maps = shard_inputs(cfg, hidden_states, cos, sin, w_qkv, w_out, H, KV)
    out, _ = run(cfg, in_maps)
    return out.reshape(B, S, D).astype(np.float32)

